# revision 11
# baseline (speedup 1.0000x reference)
"""Trainium2 Bass kernel: causal multi-head self-attention with RoPE.

Problem: x[2,2048,1024], 16 heads, d_k=64, causal, RoPE(theta=1e4),
out = (softmax(rope(Q)rope(K)^T/8) V) WO^T.

Sharding (8 cores): data-parallel over batch (2) x head-parallel over
head groups (4 heads per core).  Each core computes Q/K/V projections
for its 4 heads, flash-style causal attention, and a partial output
projection over its 256 channels; the host sums the 4 partials per
batch element.

Fully software-pipelined over 512-query slices j=0..3.  The attention
inner loop (scores -> exp -> PV) is rate-limited by the Scalar engine's
exp, so all other PE work (next slice's Q/K/V projections, previous
slice's normalize + output projection) is chopped into ~2-matmul filler
steps and interleaved between attention key-block pairs: the PE never
idles waiting on exp (keeps the HAM power state warm) and the tail of
each slice hides under the next slice's attention.

Device layouts (per core, bf16 unless noted):
  x_sb [128, 8, 2048]: x[b]^T as 8 contraction chunks of 128.
  qrot/krot [128, 2048] per head-pair: rows = [32 even-dim, 32 odd-dim]
      per head (host permutes W_Q/W_K columns) so RoPE is row-block ops.
  v_sb [128, 16, 4, 65]: [s-block, sb, head, d_v+1]; 65th col of ones
      gives softmax denominators for free from the PV matmul.
  scores transposed ([keys, queries]); causal masking by a PE-local
      eye @ (-1e5 strict-upper-triangle) accumulate onto the 128x128
      diagonal sub-block (no cross-engine dependency); fully masked
      128-col segments are skipped in scores, exp and PV.
  softmax normalization deferred: unnormalized head outputs (f32) plus
      denominator rows staged to SBUF; per-slice reciprocal + an
      indicator-matmul broadcast rescales, then the output projection.
"""

import os
import sys

for _p in ("/opt/trn_rl_repo",):
    if _p not in sys.path:
        sys.path.insert(0, _p)

import numpy as np
import ml_dtypes

BF16 = ml_dtypes.bfloat16

D = 1024
S = 2048
H = 16
DK = 64
HPC = 4          # heads per core
NCORES = 8
THETA = 10000.0

_COMPILED = {}


def _build_nc():
    import concourse.bass as bass  # noqa: F401
    import concourse.bacc as bacc
    import concourse.mybir as mybir
    import concourse.tile as tile

    bf16 = mybir.dt.bfloat16
    f32 = mybir.dt.float32
    Exp = mybir.ActivationFunctionType.Exp

    nc = bacc.Bacc(
        "TRN2", target_bir_lowering=False, debug=False, num_devices=NCORES
    )
    xt_d = nc.declare_dram_parameter("xt", [4, 128, 8, 512], bf16, isOutput=False)
    wq_d = nc.declare_dram_parameter("wq", [128, 8, 256], bf16, isOutput=False)
    wk_d = nc.declare_dram_parameter("wk", [128, 8, 256], bf16, isOutput=False)
    wv_d = nc.declare_dram_parameter("wv", [128, 8, 256], bf16, isOutput=False)
    wo_d = nc.declare_dram_parameter("wo", [128, 2, D], bf16, isOutput=False)
    cos_d = nc.declare_dram_parameter("cosb", [128, S], bf16, isOutput=False)
    sin_d = nc.declare_dram_parameter("sinb", [128, S], bf16, isOutput=False)
    eye_d = nc.declare_dram_parameter("eye", [128, 128], bf16, isOutput=False)
    trx_d = nc.declare_dram_parameter("trx", [128, 128], bf16, isOutput=False)
    ind_d = nc.declare_dram_parameter("ind", [16, 8, 128], bf16, isOutput=False)
    out_d = nc.declare_dram_parameter("out", [S, D], bf16, isOutput=True)

    with tile.TileContext(nc) as tc:
        with tc.tile_pool(name="const", bufs=1) as const:
            x_sb = const.tile([128, 8, S], bf16)
            wq_sb = const.tile([128, 8, 256], bf16)
            wk_sb = const.tile([128, 8, 256], bf16)
            wv_sb = const.tile([128, 8, 256], bf16)
            wo_sb = const.tile([128, 2, D], bf16)
            cos_sb = const.tile([128, S], bf16)
            sin_sb = const.tile([128, S], bf16)
            eye_sb = const.tile([128, 128], bf16)
            trx_sb = const.tile([128, 128], bf16)
            ind_sb = const.tile([16, 8, 128], bf16)
            v_sb = const.tile([128, 16, 4, 65], bf16)
            qraw = [const.tile([128, S], bf16, name=f"qraw{i}") for i in range(2)]
            kraw = [const.tile([128, S], bf16, name=f"kraw{i}") for i in range(2)]
            qrot = [const.tile([128, S], bf16, name=f"qrot{i}") for i in range(2)]
            krot = [const.tile([128, S], bf16, name=f"krot{i}") for i in range(2)]
            at = [const.tile([128, S], f32, name=f"at{i}") for i in range(2)]
            atn = [const.tile([128, 4, 512], bf16, name=f"atn{i}")
                   for i in range(2)]
            den_sb = const.tile([16, 512], f32)
            rc = const.tile([16, 512], f32)
            rcb = const.tile([16, 512], bf16)
            warm = const.tile([1, 16], f32)
            warmo = const.tile([1, 16], bf16)

            # x slices stream on the HW-DGE (sync) queue; weights and
            # constants are issued in parallel from the gpsimd queue
            for nsl in range(4):
                nc.sync.dma_start(
                    x_sb[:, :, nsl * 512:(nsl + 1) * 512], xt_d[nsl]
                )
            # wq/wk stream per contraction chunk so the first projection
            # matmuls start as soon as chunk 0 lands
            for c in range(8):
                nc.gpsimd.dma_start(wq_sb[:, c, :], wq_d[:, c, :])
            for c in range(8):
                nc.gpsimd.dma_start(wk_sb[:, c, :], wk_d[:, c, :])
            nc.gpsimd.dma_start(cos_sb[:], cos_d[:])
            nc.gpsimd.dma_start(sin_sb[:], sin_d[:])
            nc.gpsimd.dma_start(wv_sb[:], wv_d[:])
            nc.gpsimd.dma_start(eye_sb[:], eye_d[:])
            nc.gpsimd.dma_start(trx_sb[:], trx_d[:])
            nc.gpsimd.dma_start(ind_sb[:], ind_d[:])
            nc.gpsimd.dma_start(wo_sb[:], wo_d[:])
            nc.vector.memset(v_sb[:, :, :, 64:65], 1.0)
            # rbp's indicator matmul contracts over all 16 rcb rows; rows of
            # other slices must be finite when multiplied by 0.  den starts
            # at 1.0 because recip() runs full-tile (custom DVE ops cannot
            # start at a nonzero partition).
            nc.vector.memset(rcb[:], 1.0)
            nc.vector.memset(den_sb[:], 1.0)
            # load the Exp activation table off the critical path
            nc.vector.memset(warm[:], 0.0)
            nc.scalar.activation(warmo[:], warm[:], Exp)

            with tc.tile_pool(name="rp", bufs=2) as rp, \
                 tc.tile_pool(name="pjp", bufs=2, space="PSUM") as pjp, \
                 tc.tile_pool(name="spp", bufs=2, space="PSUM") as spp, \
                 tc.tile_pool(name="pop", bufs=2, space="PSUM") as pop, \
                 tc.tile_pool(name="ptp", bufs=3) as ptp, \
                 tc.tile_pool(name="tmp", bufs=3) as tmp, \
                 tc.tile_pool(name="obp", bufs=4) as obp:

                def qk_steps(j, t):
                    # t in 0..3 -> (wq ot0, wq ot1, wk ot0, wk ot1):
                    # project + rope one 128-row tile for query slice j,
                    # chopped into 2-matmul filler steps.
                    w_sb = wq_sb if t < 2 else wk_sb
                    raw = qraw if t < 2 else kraw
                    rot = qrot if t < 2 else krot
                    ot = t % 2
                    c0, c1 = j * 512, (j + 1) * 512
                    hold = {}

                    def mmstep(cc, hold=hold):
                        if cc == 0:
                            hold["ps"] = pjp.tile([128, 512], f32,
                                                  tag="pj", name="pj")
                        ps = hold["ps"]
                        for c in (cc, cc + 1):
                            nc.tensor.matmul(
                                ps[:],
                                w_sb[:, c, ot * 128:(ot + 1) * 128],
                                x_sb[:, c, c0:c1],
                                start=(c == 0), stop=(c == 7),
                            )
                        if cc == 6:
                            nc.vector.tensor_copy(raw[ot][:, c0:c1], ps[:])
                            sw = rp.tile([128, 512], bf16, tag="sw", name="sw")
                            hold["sw"] = sw
                            for blk in range(4):
                                src = blk ^ 1
                                nc.sync.dma_start(
                                    sw[blk * 32:(blk + 1) * 32, :],
                                    raw[ot][src * 32:(src + 1) * 32, c0:c1],
                                )

                    def ropestep(hold=hold):
                        sw = hold["sw"]
                        t1 = rp.tile([128, 512], bf16, tag="t1", name="t1")
                        nc.vector.tensor_mul(
                            t1[:], raw[ot][:, c0:c1], cos_sb[:, c0:c1]
                        )
                        nc.vector.tensor_mul(sw[:], sw[:], sin_sb[:, c0:c1])
                        nc.vector.tensor_add(rot[ot][:, c0:c1], t1[:], sw[:])

                    return [lambda cc=cc: mmstep(cc) for cc in (0, 2, 4, 6)] + \
                           [ropestep]

                def v_steps(sb):
                    hold = {}

                    def mmstep(cc, hold=hold):
                        if cc == 0:
                            hold["ps"] = pjp.tile([128, 256], f32,
                                                  tag="pj", name="pv")
                        ps = hold["ps"]
                        for c in (cc, cc + 1):
                            nc.tensor.matmul(
                                ps[:],
                                x_sb[:, c, sb * 128:(sb + 1) * 128],
                                wv_sb[:, c, :],
                                start=(c == 0), stop=(c == 7),
                            )
                        if cc == 6:
                            nc.vector.tensor_copy(
                                v_sb[:, sb, :, 0:64],
                                ps[:].rearrange("p (h d) -> p h d", h=4),
                            )

                    return [lambda cc=cc: mmstep(cc) for cc in (0, 2, 4, 6)]

                def rbp_atn_step(j, ot):
                    rbp = pjp.tile([128, 512], f32, tag="pj", name="rb")
                    nc.tensor.matmul(
                        rbp[:], ind_sb[:, j * 2 + ot, :], rcb[:],
                        start=True, stop=True,
                    )
                    nc.vector.tensor_mul(
                        atn[ot][:, j, :],
                        at[ot][:, j * 512:(j + 1) * 512],
                        rbp[:],
                    )

                def outproj_step(j, sbi, osl):
                    sb = j * 4 + sbi
                    pf = pjp.tile([128, 512], f32, tag="pj", name="pf")
                    for ich in range(2):
                        nc.tensor.matmul(
                            pf[:],
                            atn[ich][:, j, sbi * 128:(sbi + 1) * 128],
                            wo_sb[:, ich, osl * 512:(osl + 1) * 512],
                            start=(ich == 0), stop=(ich == 1),
                        )
                    ob = obp.tile([128, 512], bf16, tag="ob", name="ob")
                    if osl == 0:
                        nc.scalar.copy(ob[:], pf[:])
                    else:
                        nc.vector.tensor_copy(ob[:], pf[:])
                    # out goes on the gpsimd queue, keeping sync free for
                    # the latency-critical swap/at/den staging
                    nc.gpsimd.dma_start(
                        out_d[sb * 128:(sb + 1) * 128,
                              osl * 512:(osl + 1) * 512],
                        ob[:],
                    )

                def tail_steps(j):
                    steps = []
                    for ot in range(2):
                        steps.append(lambda j=j, ot=ot: rbp_atn_step(j, ot))
                    for sbi in range(4):
                        for osl in range(2):
                            steps.append(
                                lambda j=j, sbi=sbi, osl=osl:
                                outproj_step(j, sbi, osl)
                            )
                    return steps

                def attn_head(j, h, filler, pops=1):
                    ot, hl = divmod(h, 2)
                    r0 = hl * 64
                    qr, kr = qrot[ot], krot[ot]
                    nkb = 4 * (j + 1)
                    po = pop.tile([65, 512], f32, tag="po", name="po")
                    for kbp in range(nkb // 2):
                        kb0 = 2 * kbp
                        sp = spp.tile([128, 1024], f32, tag="sp", name="sp")
                        pt = ptp.tile([128, 1024], bf16, tag="pt", name="pt")
                        for i in range(2):
                            kb = kb0 + i
                            dg = kb - 4 * j
                            c0 = dg * 128 if dg > 0 else 0
                            diag = dg >= 0
                            nc.tensor.matmul(
                                sp[:, i * 512 + c0:(i + 1) * 512],
                                kr[r0:r0 + 64, kb * 128:(kb + 1) * 128],
                                qr[r0:r0 + 64, j * 512 + c0:(j + 1) * 512],
                                start=True, stop=not diag,
                            )
                            if diag:
                                # causal mask: accumulate -1e5 onto the
                                # 128x128 diagonal sub-block, PE-local
                                a = i * 512 + c0
                                nc.tensor.matmul(
                                    sp[:, a:a + 128],
                                    eye_sb[:],
                                    trx_sb[:],
                                    start=False, stop=True,
                                    skip_group_check=True,
                                )
                        # fully masked leading cols are never read downstream:
                        # exp may cover stale psum there (cheaper than 2 instrs)
                        cs = (kb0 - 4 * j) * 128 if kb0 > 4 * j else 0
                        nc.scalar.activation(
                            pt[:, cs:1024], sp[:, cs:1024], Exp, scale=0.125
                        )
                        for i in range(2):
                            kb = kb0 + i
                            dg = kb - 4 * j
                            c0 = dg * 128 if dg > 0 else 0
                            nc.tensor.matmul(
                                po[:, c0:512],
                                v_sb[:, kb, h, 0:65],
                                pt[:, i * 512 + c0:(i + 1) * 512],
                                start=(kb == 0), stop=(kb == nkb - 1),
                            )
                        for _ in range(pops):
                            if filler:
                                filler.pop(0)()
                    tm = tmp.tile([65, 512], f32, tag="tm", name="tm")
                    nc.vector.tensor_copy(tm[:], po[:])
                    nc.sync.dma_start(
                        at[ot][r0:r0 + 64, j * 512:(j + 1) * 512], tm[0:64, :]
                    )
                    dr = j * 4 + ot * 2 + hl
                    nc.sync.dma_start(den_sb[dr:dr + 1, :], tm[64:65, :])

                def recip(j):
                    nc.vector.reciprocal_approx_fast(rc[:], den_sb[:])
                    nc.vector.tensor_copy(rcb[:], rc[:])

                # prologue: projections for slice 0.  All matmul/copy/swap
                # steps first, rope multiplies after, so the DVE queue is
                # never head-of-line blocked on a swap DMA.
                ropes = []
                for t in range(4):
                    steps = qk_steps(0, t)
                    for st in steps[:4]:
                        st()
                    ropes.append(steps[4])
                for st in ropes:
                    st()
                for sb in range(4):
                    for st in v_steps(sb):
                        st()

                for j in range(4):
                    filler = []
                    if j < 3:
                        for t in range(4):
                            filler += qk_steps(j + 1, t)
                        if j < 2:
                            for sbi in range(4):
                                filler += v_steps(4 * (j + 1) + sbi)
                    else:
                        # slice 3's V projections run inside slice 3's
                        # attention (their key blocks are only needed from
                        # pair 6 on); 3 pops/pair keeps them ahead
                        for sbi in range(4):
                            filler += v_steps(12 + sbi)
                    if j > 0:
                        filler += tail_steps(j - 1)
                    # filler steps are emitted after each attention pair;
                    # leftovers drain proportionally at head boundaries
                    total = len(filler)
                    for h in range(HPC):
                        attn_head(j, h, filler, pops=3 if j == 3 else 1)
                        if h == 1:
                            recip(j)   # (j, ot0) denominator rows are ready
                        target = total * (HPC - 1 - h) // HPC
                        while len(filler) > target:
                            filler.pop(0)()
                    recip(j)
                # epilogue: tail of the last slice
                for st in tail_steps(3):
                    st()
    nc.compile()
    return nc


def _host_prep(x, token_positions, WQ, WK, WV, WO):
    """Build the 8 per-core input maps."""
    pos = np.asarray(token_positions).astype(np.float32)
    k = np.arange(DK // 2, dtype=np.float32)
    inv_freq = 1.0 / (THETA ** (2.0 * k / DK))
    ang = pos[:, None] * inv_freq[None, :]          # [S, 32]
    c32 = np.cos(ang).T.astype(np.float32)          # [32, S]
    s32 = np.sin(ang).T.astype(np.float32)
    cosb = np.tile(c32, (4, 1)).astype(BF16)        # [128, S]
    sinb = np.concatenate([-s32, s32, -s32, s32], axis=0).astype(BF16)
    eye = np.eye(128, dtype=np.float32).astype(BF16)
    # -1e5 on the strict upper triangle (key > query) of the 128x128
    # diagonal sub-block
    kk = np.arange(128)[:, None]
    qq = np.arange(128)[None, :]
    trx = np.where(kk > qq, -1e5, 0.0).astype(np.float32).astype(BF16)
    # indicator matrices for the denominator broadcast:
    # ind[i, j*2+ot, r] = 1 iff i == j*4 + ot*2 + r//64
    ind = np.zeros((16, 8, 128), dtype=np.float32)
    for j in range(4):
        for ot in range(2):
            for r in range(128):
                ind[j * 4 + ot * 2 + r // 64, j * 2 + ot, r] = 1.0
    ind = ind.astype(BF16)

    perm = np.concatenate([np.arange(0, DK, 2), np.arange(1, DK, 2)])  # evens,odds

    in_maps = []
    for core in range(NCORES):
        b, hg = divmod(core, 4)
        ch0 = hg * 256
        qk_rows = np.concatenate([ch0 + hl * 64 + perm for hl in range(HPC)])
        def dev_w(w):  # [D, M] -> [128, 8, M] (contraction chunks)
            return np.ascontiguousarray(
                w.reshape(8, 128, -1).transpose(1, 0, 2)
            ).astype(BF16)

        xt = np.asarray(x[b]).T                       # [D, S]
        xt4 = np.ascontiguousarray(
            xt.reshape(8, 128, 4, 512).transpose(2, 1, 0, 3)
        ).astype(BF16)                                # [4, 128, 8, 512]
        in_maps.append({
            "xt": xt4,
            "wq": dev_w(np.asarray(WQ)[qk_rows, :].T),
            "wk": dev_w(np.asarray(WK)[qk_rows, :].T),
            "wv": dev_w(np.asarray(WV)[ch0:ch0 + 256, :].T),
            "wo": np.ascontiguousarray(
                np.asarray(WO)[:, ch0:ch0 + 256].T.reshape(2, 128, D)
                .transpose(1, 0, 2)
            ).astype(BF16),
            "cosb": cosb,
            "sinb": sinb,
            "eye": eye,
            "trx": trx,
            "ind": ind,
        })
    return in_maps


LAST_EXEC_NS = None


def kernel(x, token_positions, WQ, WK, WV, WO):
    global LAST_EXEC_NS
    from concourse.bass_utils import run_bass_kernel_spmd

    if "nc" not in _COMPILED:
        _COMPILED["nc"] = _build_nc()
    nc = _COMPILED["nc"]

    in_maps = _host_prep(x, token_positions, WQ, WK, WV, WO)
    res = run_bass_kernel_spmd(nc, in_maps, list(range(NCORES)))
    LAST_EXEC_NS = res.exec_time_ns

    out = np.zeros((2, S, D), dtype=np.float32)
    for core in range(NCORES):
        out[core // 4] += np.asarray(res.results[core]["out"], dtype=np.float32)
    return out


# revision 15
# speedup vs baseline: 1.0196x; 1.0196x over previous
"""Trainium2 Bass kernel: causal multi-head self-attention with RoPE.

Problem: x[2,2048,1024], 16 heads, d_k=64, causal, RoPE(theta=1e4),
out = (softmax(rope(Q)rope(K)^T/8) V) WO^T.

Sharding (8 cores): data-parallel over batch (2) x head-parallel over
head groups (4 heads per core).  Each core computes Q/K/V projections
for its 4 heads, flash-style causal attention, and a partial output
projection over its 256 channels; the host sums the 4 partials per
batch element.

Fully software-pipelined over 512-query slices j=0..3.  The attention
inner loop (scores -> exp -> PV) is rate-limited by the Scalar engine's
exp, so all other PE work (next slice's Q/K/V projections, previous
slice's normalize + output projection) is chopped into ~2-matmul filler
steps and interleaved between attention key-block pairs: the PE never
idles waiting on exp (keeps the HAM power state warm) and the tail of
each slice hides under the next slice's attention.

Device layouts (per core, bf16 unless noted):
  x_sb [128, 8, 2048]: x[b]^T as 8 contraction chunks of 128.
  qrot/krot [128, 2048] per head-pair: rows = [32 even-dim, 32 odd-dim]
      per head (host permutes W_Q/W_K columns) so RoPE is row-block ops.
  v_sb [128, 16, 4, 65]: [s-block, sb, head, d_v+1]; 65th col of ones
      gives softmax denominators for free from the PV matmul.
  scores transposed ([keys, queries]); causal masking by a PE-local
      eye @ (-1e5 strict-upper-triangle) accumulate onto the 128x128
      diagonal sub-block (no cross-engine dependency); fully masked
      128-col segments are skipped in scores, exp and PV.
  softmax normalization deferred: unnormalized head outputs (f32) plus
      denominator rows staged to SBUF; per-slice reciprocal + an
      indicator-matmul broadcast rescales, then the output projection.
"""

import os
import sys

for _p in ("/opt/trn_rl_repo",):
    if _p not in sys.path:
        sys.path.insert(0, _p)

import numpy as np
import ml_dtypes

BF16 = ml_dtypes.bfloat16

D = 1024
S = 2048
H = 16
DK = 64
HPC = 4          # heads per core
NCORES = 8
THETA = 10000.0

_COMPILED = {}


def _build_nc():
    import concourse.bass as bass  # noqa: F401
    import concourse.bacc as bacc
    import concourse.mybir as mybir
    import concourse.tile as tile

    bf16 = mybir.dt.bfloat16
    f32 = mybir.dt.float32
    Exp = mybir.ActivationFunctionType.Exp

    nc = bacc.Bacc(
        "TRN2", target_bir_lowering=False, debug=False, num_devices=NCORES
    )
    xt_d = nc.declare_dram_parameter("xt", [4, 128, 8, 512], bf16, isOutput=False)
    wq_d = nc.declare_dram_parameter("wq", [128, 8, 256], bf16, isOutput=False)
    wk_d = nc.declare_dram_parameter("wk", [128, 8, 256], bf16, isOutput=False)
    wv_d = nc.declare_dram_parameter("wv", [128, 8, 256], bf16, isOutput=False)
    wo_d = nc.declare_dram_parameter("wo", [128, 2, D], bf16, isOutput=False)
    cos_d = nc.declare_dram_parameter("cosb", [128, S], bf16, isOutput=False)
    sin_d = nc.declare_dram_parameter("sinb", [128, S], bf16, isOutput=False)
    eye_d = nc.declare_dram_parameter("eye", [128, 128], bf16, isOutput=False)
    trx_d = nc.declare_dram_parameter("trx", [128, 128], bf16, isOutput=False)
    ind_d = nc.declare_dram_parameter("ind", [16, 8, 128], bf16, isOutput=False)
    out_d = nc.declare_dram_parameter("out", [S, D], bf16, isOutput=True)

    with tile.TileContext(nc) as tc:
        with tc.tile_pool(name="const", bufs=1) as const:
            x_sb = const.tile([128, 8, S], bf16)
            wq_sb = const.tile([128, 8, 256], bf16)
            wk_sb = const.tile([128, 8, 256], bf16)
            wv_sb = const.tile([128, 8, 256], bf16)
            wo_sb = const.tile([128, 2, D], bf16)
            cos_sb = const.tile([128, S], bf16)
            sin_sb = const.tile([128, S], bf16)
            eye_sb = const.tile([128, 128], bf16)
            trx_sb = const.tile([128, 128], bf16)
            ind_sb = const.tile([16, 8, 128], bf16)
            v_sb = const.tile([128, 16, 4, 65], bf16)
            qraw = [const.tile([128, S], bf16, name=f"qraw{i}") for i in range(2)]
            kraw = [const.tile([128, S], bf16, name=f"kraw{i}") for i in range(2)]
            qrot = [const.tile([128, S], bf16, name=f"qrot{i}") for i in range(2)]
            krot = [const.tile([128, S], bf16, name=f"krot{i}") for i in range(2)]
            at = [const.tile([128, S], f32, name=f"at{i}") for i in range(2)]
            atn = [const.tile([128, 4, 512], bf16, name=f"atn{i}")
                   for i in range(2)]
            den_sb = const.tile([16, 512], f32)
            rc = const.tile([16, 512], f32)
            rcb = const.tile([16, 512], bf16)
            warm = const.tile([1, 16], f32)
            warmo = const.tile([1, 16], bf16)

            # x chunk 0 streams first on the HW-DGE (sync) queue — chunks
            # 1-3 are issued after the prologue's swap DMAs so the swaps
            # (which gate RoPE and thus attention start) aren't queued
            # behind 8MB of x.  Weights go in parallel on the gpsimd queue.
            nc.sync.dma_start(x_sb[:, :, 0:512], xt_d[0])
            nc.gpsimd.dma_start(wq_sb[:], wq_d[:])
            nc.gpsimd.dma_start(wk_sb[:], wk_d[:])
            nc.gpsimd.dma_start(cos_sb[:], cos_d[:])
            nc.gpsimd.dma_start(sin_sb[:], sin_d[:])
            nc.gpsimd.dma_start(wv_sb[:], wv_d[:])
            nc.gpsimd.dma_start(eye_sb[:], eye_d[:])
            nc.gpsimd.dma_start(trx_sb[:], trx_d[:])
            nc.gpsimd.dma_start(ind_sb[:], ind_d[:])
            nc.gpsimd.dma_start(wo_sb[:], wo_d[:])
            nc.vector.memset(v_sb[:, :, :, 64:65], 1.0)
            # rbp's indicator matmul contracts over all 16 rcb rows; rows of
            # other slices must be finite when multiplied by 0.  den starts
            # at 1.0 because recip() runs full-tile (custom DVE ops cannot
            # start at a nonzero partition).
            nc.vector.memset(rcb[:], 1.0)
            nc.vector.memset(den_sb[:], 1.0)
            # load the Exp activation table off the critical path
            nc.vector.memset(warm[:], 0.0)
            nc.scalar.activation(warmo[:], warm[:], Exp)

            with tc.tile_pool(name="rp", bufs=2) as rp, \
                 tc.tile_pool(name="pjp", bufs=2, space="PSUM") as pjp, \
                 tc.tile_pool(name="spp", bufs=2, space="PSUM") as spp, \
                 tc.tile_pool(name="pop", bufs=2, space="PSUM") as pop, \
                 tc.tile_pool(name="ptp", bufs=3) as ptp, \
                 tc.tile_pool(name="tmp", bufs=3) as tmp, \
                 tc.tile_pool(name="obp", bufs=4) as obp:

                def qk_steps(j, t):
                    # t in 0..3 -> (wq ot0, wq ot1, wk ot0, wk ot1):
                    # project + rope one 128-row tile for query slice j,
                    # chopped into 2-matmul filler steps.
                    w_sb = wq_sb if t < 2 else wk_sb
                    raw = qraw if t < 2 else kraw
                    rot = qrot if t < 2 else krot
                    ot = t % 2
                    c0, c1 = j * 512, (j + 1) * 512
                    hold = {}

                    def mmstep(cc, hold=hold):
                        if cc == 0:
                            hold["ps"] = pjp.tile([128, 512], f32,
                                                  tag="pj", name="pj")
                        ps = hold["ps"]
                        for c in (cc, cc + 1):
                            nc.tensor.matmul(
                                ps[:],
                                w_sb[:, c, ot * 128:(ot + 1) * 128],
                                x_sb[:, c, c0:c1],
                                start=(c == 0), stop=(c == 7),
                            )
                        if cc == 6:
                            nc.vector.tensor_copy(raw[ot][:, c0:c1], ps[:])
                            sw = rp.tile([128, 512], bf16, tag="sw", name="sw")
                            hold["sw"] = sw
                            for blk in range(4):
                                src = blk ^ 1
                                nc.sync.dma_start(
                                    sw[blk * 32:(blk + 1) * 32, :],
                                    raw[ot][src * 32:(src + 1) * 32, c0:c1],
                                )

                    def ropestep(hold=hold):
                        sw = hold["sw"]
                        t1 = rp.tile([128, 512], bf16, tag="t1", name="t1")
                        nc.vector.tensor_mul(
                            t1[:], raw[ot][:, c0:c1], cos_sb[:, c0:c1]
                        )
                        nc.vector.tensor_mul(sw[:], sw[:], sin_sb[:, c0:c1])
                        nc.vector.tensor_add(rot[ot][:, c0:c1], t1[:], sw[:])

                    return [lambda cc=cc: mmstep(cc) for cc in (0, 2, 4, 6)] + \
                           [ropestep]

                def v_steps(sb):
                    hold = {}

                    def mmstep(cc, hold=hold):
                        if cc == 0:
                            hold["ps"] = pjp.tile([128, 256], f32,
                                                  tag="pj", name="pv")
                        ps = hold["ps"]
                        for c in (cc, cc + 1):
                            nc.tensor.matmul(
                                ps[:],
                                x_sb[:, c, sb * 128:(sb + 1) * 128],
                                wv_sb[:, c, :],
                                start=(c == 0), stop=(c == 7),
                            )
                        if cc == 6:
                            nc.vector.tensor_copy(
                                v_sb[:, sb, :, 0:64],
                                ps[:].rearrange("p (h d) -> p h d", h=4),
                            )

                    return [lambda cc=cc: mmstep(cc) for cc in (0, 2, 4, 6)]

                def rbp_atn_step(j, ot):
                    rbp = pjp.tile([128, 512], f32, tag="pj", name="rb")
                    nc.tensor.matmul(
                        rbp[:], ind_sb[:, j * 2 + ot, :], rcb[:],
                        start=True, stop=True,
                    )
                    nc.vector.tensor_mul(
                        atn[ot][:, j, :],
                        at[ot][:, j * 512:(j + 1) * 512],
                        rbp[:],
                    )

                def outproj_step(j, sbi, osl):
                    sb = j * 4 + sbi
                    pf = pjp.tile([128, 512], f32, tag="pj", name="pf")
                    for ich in range(2):
                        nc.tensor.matmul(
                            pf[:],
                            atn[ich][:, j, sbi * 128:(sbi + 1) * 128],
                            wo_sb[:, ich, osl * 512:(osl + 1) * 512],
                            start=(ich == 0), stop=(ich == 1),
                        )
                    ob = obp.tile([128, 512], bf16, tag="ob", name="ob")
                    if osl == 0:
                        nc.scalar.copy(ob[:], pf[:])
                    else:
                        nc.vector.tensor_copy(ob[:], pf[:])
                    # out goes on the gpsimd queue, keeping sync free for
                    # the latency-critical swap/at/den staging
                    nc.gpsimd.dma_start(
                        out_d[sb * 128:(sb + 1) * 128,
                              osl * 512:(osl + 1) * 512],
                        ob[:],
                    )

                def tail_steps(j):
                    steps = []
                    for ot in range(2):
                        steps.append(lambda j=j, ot=ot: rbp_atn_step(j, ot))
                    for sbi in range(4):
                        for osl in range(2):
                            steps.append(
                                lambda j=j, sbi=sbi, osl=osl:
                                outproj_step(j, sbi, osl)
                            )
                    return steps

                def attn_head(j, h, filler, pops=1):
                    ot, hl = divmod(h, 2)
                    r0 = hl * 64
                    qr, kr = qrot[ot], krot[ot]
                    nkb = 4 * (j + 1)
                    po = pop.tile([65, 512], f32, tag="po", name="po")
                    for kbp in range(nkb // 2):
                        kb0 = 2 * kbp
                        sp = spp.tile([128, 1024], f32, tag="sp", name="sp")
                        pt = ptp.tile([128, 1024], bf16, tag="pt", name="pt")
                        for i in range(2):
                            kb = kb0 + i
                            dg = kb - 4 * j
                            c0 = dg * 128 if dg > 0 else 0
                            diag = dg >= 0
                            nc.tensor.matmul(
                                sp[:, i * 512 + c0:(i + 1) * 512],
                                kr[r0:r0 + 64, kb * 128:(kb + 1) * 128],
                                qr[r0:r0 + 64, j * 512 + c0:(j + 1) * 512],
                                start=True, stop=not diag,
                            )
                            if diag:
                                # causal mask: accumulate -1e5 onto the
                                # 128x128 diagonal sub-block, PE-local
                                a = i * 512 + c0
                                nc.tensor.matmul(
                                    sp[:, a:a + 128],
                                    eye_sb[:],
                                    trx_sb[:],
                                    start=False, stop=True,
                                    skip_group_check=True,
                                )
                        # fully masked leading cols are never read downstream:
                        # exp may cover stale psum there (cheaper than 2 instrs)
                        cs = (kb0 - 4 * j) * 128 if kb0 > 4 * j else 0
                        nc.scalar.activation(
                            pt[:, cs:1024], sp[:, cs:1024], Exp, scale=0.125
                        )
                        for i in range(2):
                            kb = kb0 + i
                            dg = kb - 4 * j
                            c0 = dg * 128 if dg > 0 else 0
                            nc.tensor.matmul(
                                po[:, c0:512],
                                v_sb[:, kb, h, 0:65],
                                pt[:, i * 512 + c0:(i + 1) * 512],
                                start=(kb == 0), stop=(kb == nkb - 1),
                            )
                        for _ in range(pops):
                            if filler:
                                filler.pop(0)()
                    # den DMA first: it feeds the reciprocal chain
                    tm = tmp.tile([65, 512], f32, tag="tm", name="tm")
                    nc.vector.tensor_copy(tm[:], po[:])
                    dr = j * 4 + ot * 2 + hl
                    nc.sync.dma_start(den_sb[dr:dr + 1, :], tm[64:65, :])
                    nc.sync.dma_start(
                        at[ot][r0:r0 + 64, j * 512:(j + 1) * 512], tm[0:64, :]
                    )

                def recip(j):
                    nc.vector.reciprocal_approx_fast(rc[:], den_sb[:])
                    nc.vector.tensor_copy(rcb[:], rc[:])

                # prologue: projections for slice 0.  All matmul/copy/swap
                # steps first, rope multiplies after, so the DVE queue is
                # never head-of-line blocked on a swap DMA.
                ropes = []
                for t in range(4):
                    steps = qk_steps(0, t)
                    for st in steps[:4]:
                        st()
                    ropes.append(steps[4])
                for nsl in range(1, 4):
                    nc.sync.dma_start(
                        x_sb[:, :, nsl * 512:(nsl + 1) * 512], xt_d[nsl]
                    )
                for st in ropes:
                    st()
                for sb in range(4):
                    for st in v_steps(sb):
                        st()

                for j in range(4):
                    filler = []
                    if j < 3:
                        for t in range(4):
                            filler += qk_steps(j + 1, t)
                        if j < 2:
                            for sbi in range(4):
                                filler += v_steps(4 * (j + 1) + sbi)
                    else:
                        # slice 3's V projections run inside slice 3's
                        # attention (their key blocks are only needed from
                        # pair 6 on); 3 pops/pair keeps them ahead
                        for sbi in range(4):
                            filler += v_steps(12 + sbi)
                    if j > 0:
                        filler += tail_steps(j - 1)
                    # filler steps are emitted after each attention pair;
                    # leftovers drain proportionally at head boundaries
                    total = len(filler)
                    for h in range(HPC):
                        attn_head(j, h, filler, pops=3 if j == 3 else 1)
                        if h == 1:
                            recip(j)   # (j, ot0) denominator rows are ready
                        target = total * (HPC - 1 - h) // HPC
                        while len(filler) > target:
                            filler.pop(0)()
                    recip(j)
                # epilogue: tail of the last slice
                for st in tail_steps(3):
                    st()
    nc.compile()
    return nc


def _host_prep(x, token_positions, WQ, WK, WV, WO):
    """Build the 8 per-core input maps."""
    pos = np.asarray(token_positions).astype(np.float32)
    k = np.arange(DK // 2, dtype=np.float32)
    inv_freq = 1.0 / (THETA ** (2.0 * k / DK))
    ang = pos[:, None] * inv_freq[None, :]          # [S, 32]
    c32 = np.cos(ang).T.astype(np.float32)          # [32, S]
    s32 = np.sin(ang).T.astype(np.float32)
    cosb = np.tile(c32, (4, 1)).astype(BF16)        # [128, S]
    sinb = np.concatenate([-s32, s32, -s32, s32], axis=0).astype(BF16)
    eye = np.eye(128, dtype=np.float32).astype(BF16)
    # -1e5 on the strict upper triangle (key > query) of the 128x128
    # diagonal sub-block
    kk = np.arange(128)[:, None]
    qq = np.arange(128)[None, :]
    trx = np.where(kk > qq, -1e5, 0.0).astype(np.float32).astype(BF16)
    # indicator matrices for the denominator broadcast:
    # ind[i, j*2+ot, r] = 1 iff i == j*4 + ot*2 + r//64
    ind = np.zeros((16, 8, 128), dtype=np.float32)
    for j in range(4):
        for ot in range(2):
            for r in range(128):
                ind[j * 4 + ot * 2 + r // 64, j * 2 + ot, r] = 1.0
    ind = ind.astype(BF16)

    perm = np.concatenate([np.arange(0, DK, 2), np.arange(1, DK, 2)])  # evens,odds

    in_maps = []
    for core in range(NCORES):
        b, hg = divmod(core, 4)
        ch0 = hg * 256
        qk_rows = np.concatenate([ch0 + hl * 64 + perm for hl in range(HPC)])
        def dev_w(w):  # [D, M] -> [128, 8, M] (contraction chunks)
            return np.ascontiguousarray(
                w.reshape(8, 128, -1).transpose(1, 0, 2)
            ).astype(BF16)

        xt = np.asarray(x[b]).T                       # [D, S]
        xt4 = np.ascontiguousarray(
            xt.reshape(8, 128, 4, 512).transpose(2, 1, 0, 3)
        ).astype(BF16)                                # [4, 128, 8, 512]
        in_maps.append({
            "xt": xt4,
            "wq": dev_w(np.asarray(WQ)[qk_rows, :].T),
            "wk": dev_w(np.asarray(WK)[qk_rows, :].T),
            "wv": dev_w(np.asarray(WV)[ch0:ch0 + 256, :].T),
            "wo": np.ascontiguousarray(
                np.asarray(WO)[:, ch0:ch0 + 256].T.reshape(2, 128, D)
                .transpose(1, 0, 2)
            ).astype(BF16),
            "cosb": cosb,
            "sinb": sinb,
            "eye": eye,
            "trx": trx,
            "ind": ind,
        })
    return in_maps


LAST_EXEC_NS = None


def kernel(x, token_positions, WQ, WK, WV, WO):
    global LAST_EXEC_NS
    from concourse.bass_utils import run_bass_kernel_spmd

    if "nc" not in _COMPILED:
        _COMPILED["nc"] = _build_nc()
    nc = _COMPILED["nc"]

    in_maps = _host_prep(x, token_positions, WQ, WK, WV, WO)
    res = run_bass_kernel_spmd(nc, in_maps, list(range(NCORES)))
    LAST_EXEC_NS = res.exec_time_ns

    out = np.zeros((2, S, D), dtype=np.float32)
    for core in range(NCORES):
        out[core // 4] += np.asarray(res.results[core]["out"], dtype=np.float32)
    return out
